# revision 1
# baseline (speedup 1.0000x reference)
"""DeepSeek-V3 MoE layer on 8 Trainium2 NeuronCores.

Strategy (expert-parallel + shared-expert tensor-parallel):
  - 64 routed experts sharded 8-per-core; every core computes the dense
    per-expert gated MLP for all 256 tokens of its 8 experts in bf16 and
    accumulates  sum_e cw[t,e] * expert_e(x)[t]  into PSUM.
  - the shared GatedMLP is tensor-parallel on the intermediate dim
    (2048/8 = 256 rows per core) and accumulates into the same PSUM.
  - the noaux-tc sigmoid routing (gate matmul fp32 + grouped top-k) is
    replicated on every core; it is tiny and overlaps the expert GEMMs.
  - a ReduceScatter over the [1024, 256] partial output sums the 8 cores;
    each core returns its 128-row shard of out^T, the host concatenates
    and transposes.

Everything compute-heavy runs in bf16 (fp32 PSUM accumulation); the gate
matmul and all routing arithmetic are fp32 so the top-k selection matches
the fp32 reference exactly.

The down-projection accumulation is region-major: all 34 matmuls that
accumulate one [128, 256] output region (2 shared k-tiles + 8 experts x 4
k-tiles) are emitted as one contiguous start..stop group.  Interleaving
open accumulation groups that share a PSUM bank corrupts the early
contributions (observed on hardware); the way GEMM1 closes each group
before the next opens is the pattern that works.
"""

import sys

sys.path.insert(0, "/opt/trn_rl_repo")

import numpy as np
import ml_dtypes

import concourse.bacc as bacc
import concourse.mybir as mybir
import concourse.tile as tile
from concourse.bass_utils import run_bass_kernel_spmd

T = 256
H = 1024
E = 64
I = 512
SI = 2048
TOP_K = 6
N_GROUP = 8
TOPK_GROUP = 4
ROUTED_SCALE = 2.5
N_CORES = 8
E_LOC = E // N_CORES          # 8 experts per core
SI_LOC = SI // N_CORES        # 256 shared-intermediate rows per core
KH = H // 128                 # 8 k-tiles over hidden
KI = I // 128                 # 4 k-tiles over routed intermediate
KS = SI_LOC // 128            # 2 k-tiles over local shared intermediate
HT = H // 128                 # 8 output h-tiles

F32 = mybir.dt.float32
BF16 = mybir.dt.bfloat16
NEG = -1.0e9

_cached = None


def _build():
    nc = bacc.Bacc("TRN2", target_bir_lowering=False, debug=False, num_devices=N_CORES)

    xT_in = nc.declare_dram_parameter("xT", [H, T], F32, isOutput=False)
    xTb_in = nc.declare_dram_parameter("xTb", [128, KH * T], BF16, isOutput=False)
    gwT_in = nc.declare_dram_parameter("gwT", [H, E], F32, isOutput=False)
    eb_in = nc.declare_dram_parameter("ebias_b", [128, E], F32, isOutput=False)
    sel_in = nc.declare_dram_parameter("sel", [E, E_LOC], F32, isOutput=False)
    id_in = nc.declare_dram_parameter("ident", [128, 128], F32, isOutput=False)
    oneh_in = nc.declare_dram_parameter("oneh", [E_LOC, E_LOC * 128], F32, isOutput=False)
    w13_in = nc.declare_dram_parameter("w13T", [E_LOC, 128, KH * 2 * I], BF16, isOutput=False)
    # per-output-h-tile slabs: [ht, p(i-in-ki), e*KI*128 + ki*128 + hh]
    w2_in = nc.declare_dram_parameter("w2Q", [4, 128, 4 * KI * 512], BF16, isOutput=False)
    wsgu_in = nc.declare_dram_parameter("wsgu", [128, KH * 2 * SI_LOC], BF16, isOutput=False)
    wsd_in = nc.declare_dram_parameter("wsd", [128, KS * H], BF16, isOutput=False)
    out_p = nc.declare_dram_parameter("out", [T // N_CORES, H], BF16, isOutput=True)

    with tile.TileContext(nc) as tc:
        with (
            tc.tile_pool(name="sbuf", bufs=1) as sbuf,
            tc.tile_pool(name="wpool", bufs=4) as wpool,
            tc.tile_pool(name="w2pool", bufs=4) as w2pool,
            tc.tile_pool(name="spsum", bufs=2, space="PSUM") as spsum,
            tc.tile_pool(name="hpsum", bufs=2, space="PSUM") as hpsum,
            tc.tile_pool(name="opsum", bufs=1, space="PSUM") as opsum,
            tc.tile_pool(name="dram", bufs=1, space="DRAM") as dram,
        ):
            # ---- collective warm-up: tiny RS with no compute deps; pays the
            # per-execution collective bring-up + absorbs cross-core launch
            # skew while the real work happens.  Its result is written into a
            # corner of the output (before the real output DMA) so it is
            # never dead code.
            pre_sb = sbuf.tile([16, 16], BF16)
            nc.gpsimd.memset(pre_sb[:], 0.0)
            pre_in = dram.tile([16, 16], BF16)
            pre_out = dram.tile([2, 16], BF16)
            nc.gpsimd.dma_start(pre_in[:], pre_sb[:])
            nc.gpsimd.collective_compute(
                "ReduceScatter",
                mybir.AluOpType.add,
                replica_groups=[list(range(N_CORES))],
                ins=[pre_in.opt()],
                outs=[pre_out.opt()],
            )
            nc.gpsimd.dma_start(out_p[0:2, 0:16], pre_out[:])

            # ---- input loads: the early-critical tensors go first on the
            # sync ring so the bulk w13 stream does not starve them
            xT_sb = sbuf.tile([128, KH * T], F32)
            xTb = sbuf.tile([128, KH * T], BF16)
            xv = xT_in.ap().rearrange("(k p) t -> p k t", p=128)
            x3 = xT_sb[:].rearrange("p (k t) -> p k t", k=KH)
            wsgu_sb = sbuf.tile([128, KH * 2 * SI_LOC], BF16)
            half = KH * SI_LOC
            nc.sync.dma_start(xTb[:, 0 : 4 * T], xTb_in[:, 0 : 4 * T])
            nc.sync.dma_start(wsgu_sb[:, 0:half], wsgu_in[:, 0:half])
            nc.sync.dma_start(xTb[:, 4 * T :], xTb_in[:, 4 * T :])
            nc.sync.dma_start(wsgu_sb[:, half:], wsgu_in[:, half:])
            gwT_sb = sbuf.tile([128, KH * E], F32)
            nc.sync.dma_start(
                gwT_sb[:].rearrange("p (k e) -> p k e", k=KH),
                gwT_in.ap().rearrange("(k p) e -> p k e", p=128),
            )
            for c in range(4):
                nc.sync.dma_start(x3[:, 2 * c : 2 * c + 2, :], xv[:, 2 * c : 2 * c + 2, :])
            wsd_sb = sbuf.tile([128, KS * H], BF16)
            nc.sync.dma_start(wsd_sb[:], wsd_in[:])
            eb_sb = sbuf.tile([128, E], F32)
            nc.scalar.dma_start(eb_sb[:], eb_in[:])
            sel_sb = sbuf.tile([E, E_LOC], F32)
            nc.scalar.dma_start(sel_sb[:], sel_in[:])
            id_sb = sbuf.tile([128, 128], F32)
            nc.scalar.dma_start(id_sb[:], id_in[:])
            oneh_sb = sbuf.tile([E_LOC, E_LOC * 128], F32)
            nc.scalar.dma_start(oneh_sb[:], oneh_in[:])

            # ---- shared expert gate/up GEMMs (PE keeps busy while routing
            # runs on DVE)
            su = []
            for si in range(KS):
                sp = hpsum.tile([128, 2 * T], F32, tag="h13", name=f"su{si}")
                for k in range(KH):
                    nc.tensor.matmul(
                        sp[:, 0:T],
                        wsgu_sb[:, k * 2 * SI_LOC + si * 128 : k * 2 * SI_LOC + si * 128 + 128],
                        xTb[:, k * T : (k + 1) * T],
                        start=(k == 0),
                        stop=(k == KH - 1),
                    )
                for k in range(KH):
                    nc.tensor.matmul(
                        sp[:, T : 2 * T],
                        wsgu_sb[:, k * 2 * SI_LOC + SI_LOC + si * 128 : k * 2 * SI_LOC + SI_LOC + si * 128 + 128],
                        xTb[:, k * T : (k + 1) * T],
                        start=(k == 0),
                        stop=(k == KH - 1),
                    )
                su.append(sp)

            # ---- gate matmul (fp32): logits [t, e] per 128-token tile
            logits = []
            for tt in range(2):
                lp = spsum.tile([128, E], F32, tag="small", name=f"logits{tt}")
                for k in range(KH):
                    nc.tensor.matmul(
                        lp[:],
                        xT_sb[:, k * T + tt * 128 : k * T + tt * 128 + 128],
                        gwT_sb[:, k * E : (k + 1) * E],
                        start=(k == 0),
                        stop=(k == KH - 1),
                    )
                logits.append(lp)

            # ---- routing (DVE + ACT, fp32) — replicated on every core
            cw_sb = sbuf.tile([128, 2 * E], F32)
            cwT_sb = sbuf.tile([E, T], F32)
            for tt in range(2):
                scores = sbuf.tile([128, E], F32, tag="scores")
                nc.scalar.activation(scores[:], logits[tt][:], mybir.ActivationFunctionType.Sigmoid)
                swb = sbuf.tile([128, E], F32, tag="swb")
                nc.vector.tensor_add(swb[:], scores[:], eb_sb[:])
                swb3 = swb[:].rearrange("p (g j) -> p g j", g=N_GROUP)
                m1 = sbuf.tile([128, N_GROUP], F32, tag="m1")
                nc.vector.reduce_max(m1[:], swb3, axis=mybir.AxisListType.X)
                eqt = sbuf.tile([128, E], F32, tag="eqt")
                nc.vector.tensor_tensor(
                    eqt[:].rearrange("p (g j) -> p g j", g=N_GROUP),
                    swb3,
                    m1[:].to_broadcast((128, N_GROUP, E // N_GROUP)),
                    op=mybir.AluOpType.is_equal,
                )
                swb2 = sbuf.tile([128, E], F32, tag="swb2")
                nc.vector.scalar_tensor_tensor(
                    swb2[:], eqt[:], NEG, swb[:],
                    op0=mybir.AluOpType.mult, op1=mybir.AluOpType.add,
                )
                m2 = sbuf.tile([128, N_GROUP], F32, tag="m2")
                nc.vector.reduce_max(
                    m2[:], swb2[:].rearrange("p (g j) -> p g j", g=N_GROUP),
                    axis=mybir.AxisListType.X,
                )
                gsum = sbuf.tile([128, N_GROUP], F32, tag="gsum")
                nc.vector.tensor_add(gsum[:], m1[:], m2[:])
                gmask = sbuf.tile([128, N_GROUP], F32, tag="gmask")
                nc.vector.memset(gmask[:], 0.0)
                for _ in range(TOPK_GROUP):
                    gm = sbuf.tile([128, 1], F32, tag="gm")
                    nc.vector.reduce_max(gm[:], gsum[:], axis=mybir.AxisListType.X)
                    geq = sbuf.tile([128, N_GROUP], F32, tag="geq")
                    nc.vector.tensor_scalar(geq[:], gsum[:], gm[:], None, op0=mybir.AluOpType.is_equal)
                    nc.vector.tensor_add(gmask[:], gmask[:], geq[:])
                    nc.vector.scalar_tensor_tensor(
                        gsum[:], geq[:], NEG, gsum[:],
                        op0=mybir.AluOpType.mult, op1=mybir.AluOpType.add,
                    )
                swbm = sbuf.tile([128, E], F32, tag="swbm")
                nc.vector.tensor_tensor(
                    swbm[:].rearrange("p (g j) -> p g j", g=N_GROUP),
                    swb3,
                    gmask[:].to_broadcast((128, N_GROUP, E // N_GROUP)),
                    op=mybir.AluOpType.mult,
                )
                nmask = sbuf.tile([128, E], F32, tag="nmask")
                nc.vector.memset(nmask[:], 0.0)
                for _ in range(TOP_K):
                    em = sbuf.tile([128, 1], F32, tag="em")
                    nc.vector.reduce_max(em[:], swbm[:], axis=mybir.AxisListType.X)
                    eeq = sbuf.tile([128, E], F32, tag="eeq")
                    nc.vector.tensor_scalar(eeq[:], swbm[:], em[:], None, op0=mybir.AluOpType.is_equal)
                    nc.vector.tensor_add(nmask[:], nmask[:], eeq[:])
                    nc.vector.scalar_tensor_tensor(
                        swbm[:], eeq[:], NEG, swbm[:],
                        op0=mybir.AluOpType.mult, op1=mybir.AluOpType.add,
                    )
                s_sb = sbuf.tile([128, E], F32, tag="s_sb")
                nc.vector.tensor_mul(s_sb[:], scores[:], nmask[:])
                denom = sbuf.tile([128, 1], F32, tag="denom")
                nc.vector.reduce_sum(denom[:], s_sb[:], axis=mybir.AxisListType.X)
                dr = sbuf.tile([128, 1], F32, tag="dr")
                nc.vector.reciprocal(dr[:], denom[:])
                nc.vector.tensor_scalar(
                    cw_sb[:, tt * E : (tt + 1) * E], s_sb[:], dr[:], ROUTED_SCALE,
                    op0=mybir.AluOpType.mult, op1=mybir.AluOpType.mult,
                )

            # ---- shared expert activation (ACT + DVE)
            acts_sh = sbuf.tile([128, KS * T], BF16)
            for si in range(KS):
                ssl = sbuf.tile([128, T], BF16, tag="ssl")
                nc.scalar.activation(ssl[:], su[si][:, 0:T], mybir.ActivationFunctionType.Silu)
                nc.vector.tensor_mul(acts_sh[:, si * T : (si + 1) * T], ssl[:], su[si][:, T : 2 * T])

            # ---- per-expert combine weights: cb[j] = broadcast of
            # cw[:, core*8+j] across all 128 partitions
            for tt in range(2):
                ctp = spsum.tile([E, 128], F32, tag="small", name=f"ctp{tt}")
                nc.tensor.transpose(ctp[:], cw_sb[:, tt * E : (tt + 1) * E], id_sb[:])
                nc.vector.tensor_copy(cwT_sb[:, tt * 128 : (tt + 1) * 128], ctp[:])
            cwl_ps = spsum.tile([E_LOC, T], F32, tag="small")
            nc.tensor.matmul(cwl_ps[:], sel_sb[:], cwT_sb[:], start=True, stop=True)
            cwl_sb = sbuf.tile([E_LOC, T], F32)
            nc.vector.tensor_copy(cwl_sb[:], cwl_ps[:])
            cb_sb = sbuf.tile([128, E_LOC * T], BF16)
            for j in range(E_LOC):
                cbp = spsum.tile([128, T], F32, tag="small", name=f"cbp{j}")
                nc.tensor.matmul(
                    cbp[:], oneh_sb[:, j * 128 : (j + 1) * 128], cwl_sb[:],
                    start=True, stop=True,
                )
                nc.vector.tensor_copy(cb_sb[:, j * T : (j + 1) * T], cbp[:])

            # ---- routed experts: GEMM1 + activation, all 8 acts kept in SBUF
            act_sbs = []
            for e in range(E_LOC):
                w13_sb = wpool.tile([128, KH * 2 * I], BF16, tag="w13", name=f"w13_{e}")
                for q in range(4):
                    nc.sync.dma_start(
                        w13_sb[:, q * 2 * 2 * I : (q + 1) * 2 * 2 * I],
                        w13_in[e, :, q * 2 * 2 * I : (q + 1) * 2 * 2 * I],
                    )
                act_sb = sbuf.tile([128, KI * T], BF16, tag=f"act{e}", name=f"act{e}")
                act_sbs.append(act_sb)
                for i in range(KI):
                    hp = hpsum.tile([128, 2 * T], F32, tag="h13", name=f"h13_{e}_{i}")
                    for k in range(KH):
                        nc.tensor.matmul(
                            hp[:, 0:T],
                            w13_sb[:, k * 2 * I + i * 128 : k * 2 * I + i * 128 + 128],
                            xTb[:, k * T : (k + 1) * T],
                            start=(k == 0),
                            stop=(k == KH - 1),
                        )
                    for k in range(KH):
                        nc.tensor.matmul(
                            hp[:, T : 2 * T],
                            w13_sb[:, k * 2 * I + I + i * 128 : k * 2 * I + I + i * 128 + 128],
                            xTb[:, k * T : (k + 1) * T],
                            start=(k == 0),
                            stop=(k == KH - 1),
                        )
                    sl = sbuf.tile([128, T], BF16, tag="sl")
                    nc.scalar.activation(sl[:], hp[:, 0:T], mybir.ActivationFunctionType.Silu)
                    h3s = sbuf.tile([128, T], BF16, tag="h3s")
                    nc.vector.tensor_mul(h3s[:], hp[:, T : 2 * T], cb_sb[:, e * T : (e + 1) * T])
                    nc.vector.tensor_mul(act_sb[:, i * T : (i + 1) * T], sl[:], h3s[:])

            # ---- down-projections, flipped: the act tiles are the
            # stationary operand and w2 streams as the wide (N=512) moving
            # operand, so the output comes out token-major [t, h] and
            # LDWEIGHTS hides under the 512-column stream.  4 accumulation
            # regions (tt, hh), each one closed start..stop group in its own
            # PSUM bank.
            out_ps = [opsum.tile([128, H], F32, tag=f"out{tt}", name=f"out{tt}") for tt in range(2)]
            outf = sbuf.tile([128, 2 * H], BF16)
            rs_in = dram.tile([T, H], BF16)
            rs_out = dram.tile([T // N_CORES, H], BF16)

            w2q = {}
            for q in range(4):
                w2q[q] = w2pool.tile([128, 4 * KI * 512], BF16, tag="w2q", name=f"w2q{q}")
                nc.scalar.dma_start(w2q[q][:], w2_in[q, :, :])

            for hh in range(2):
                for tt in range(2):
                    reg = out_ps[tt][:, hh * 512 : (hh + 1) * 512]
                    for ks in range(KS):
                        nc.tensor.matmul(
                            reg,
                            acts_sh[:, ks * T + tt * 128 : ks * T + tt * 128 + 128],
                            wsd_sb[:, ks * H + hh * 512 : ks * H + (hh + 1) * 512],
                            start=(ks == 0),
                            stop=False,
                        )
                    for e in range(E_LOC):
                        qt = w2q[hh * 2 + e // 4]
                        er = e % 4
                        for ki in range(KI):
                            nc.tensor.matmul(
                                reg,
                                act_sbs[e][:, ki * T + tt * 128 : ki * T + tt * 128 + 128],
                                qt[:, (er * KI + ki) * 512 : (er * KI + ki) * 512 + 512],
                                start=False,
                                stop=(e == E_LOC - 1 and ki == KI - 1),
                            )
                    if hh == 1:
                        nc.vector.tensor_copy(outf[:, tt * H : (tt + 1) * H], out_ps[tt][:])
                        nc.sync.dma_start(rs_in[tt * 128 : (tt + 1) * 128, :], outf[:, tt * H : (tt + 1) * H])

            # ---- ReduceScatter over cores: each core gets 32 tokens x H
            nc.gpsimd.collective_compute(
                "ReduceScatter",
                mybir.AluOpType.add,
                replica_groups=[list(range(N_CORES))],
                ins=[rs_in.opt()],
                outs=[rs_out.opt()],
            )
            nc.sync.dma_start(out_p[:], rs_out[:])

    nc.finalize()
    return nc


def _prep_inputs(inputs):
    bf = ml_dtypes.bfloat16
    x = np.asarray(inputs["hidden_states"], np.float32)
    gate_w = np.asarray(inputs["gate_w"], np.float32)
    e_bias = np.asarray(inputs["e_bias"], np.float32)
    w1 = np.asarray(inputs["w1"], np.float32)
    w3 = np.asarray(inputs["w3"], np.float32)
    w2 = np.asarray(inputs["w2"], np.float32)
    ws_gate = np.asarray(inputs["ws_gate"], np.float32)
    ws_up = np.asarray(inputs["ws_up"], np.float32)
    ws_down = np.asarray(inputs["ws_down"], np.float32)

    xT = np.ascontiguousarray(x.T)
    xTb = np.ascontiguousarray(x.T.reshape(KH, 128, T).transpose(1, 0, 2).reshape(128, KH * T)).astype(bf)
    gwT = np.ascontiguousarray(gate_w.T)
    ebb = np.broadcast_to(e_bias[None, :], (128, E)).copy()
    ident = np.eye(128, dtype=np.float32)
    oneh = np.zeros((E_LOC, E_LOC * 128), np.float32)
    for j in range(E_LOC):
        oneh[j, j * 128 : (j + 1) * 128] = 1.0

    # routed up/gate weights: [E, k, p, ...] -> [E, p, k*...]
    w1t = w1.transpose(0, 2, 1).reshape(E, KH, 128, I)
    w3t = w3.transpose(0, 2, 1).reshape(E, KH, 128, I)
    w13 = np.concatenate([w1t, w3t], axis=-1)          # [E, KH, 128, 2I]
    w13 = w13.transpose(0, 2, 1, 3).reshape(E, 128, KH * 2 * I).astype(bf)
    # routed down weights as rhs quarters:
    # w2Q[c][hh*2+eh, p, ((er*KI)+ki)*512 + hc] = w2[8c+4*eh+er][hh*512+hc, ki*128+p]
    w2t = w2.transpose(0, 2, 1).reshape(E, KI, 128, 2, 512)   # [e, ki, p, hh, hc]
    w2t = w2t.transpose(0, 3, 2, 1, 4)                        # [e, hh, p, ki, hc]

    in_maps = []
    for c in range(N_CORES):
        sel = np.zeros((E, E_LOC), np.float32)
        for j in range(E_LOC):
            sel[c * E_LOC + j, j] = 1.0
        wsg = ws_gate[c * SI_LOC : (c + 1) * SI_LOC, :].T.reshape(KH, 128, SI_LOC)
        wsu = ws_up[c * SI_LOC : (c + 1) * SI_LOC, :].T.reshape(KH, 128, SI_LOC)
        wsgu = np.concatenate([wsg, wsu], axis=-1).transpose(1, 0, 2).reshape(128, KH * 2 * SI_LOC).astype(bf)
        wsd = ws_down[:, c * SI_LOC : (c + 1) * SI_LOC].T.reshape(KS, 128, H)
        wsd = wsd.transpose(1, 0, 2).reshape(128, KS * H).astype(bf)
        wc = w2t[c * E_LOC : (c + 1) * E_LOC]                 # [8, hh, p, ki, hc]
        wc = wc.reshape(2, 4, 2, 128, KI, 512)                # [eh, er, hh, p, ki, hc]
        wc = wc.transpose(2, 0, 3, 1, 4, 5)                   # [hh, eh, p, er, ki, hc]
        w2r = np.ascontiguousarray(wc.reshape(4, 128, 4 * KI * 512)).astype(bf)
        in_maps.append(
            {
                "xT": xT,
                "xTb": xTb,
                "gwT": gwT,
                "ebias_b": ebb,
                "sel": sel,
                "ident": ident,
                "oneh": oneh,
                "w13T": np.ascontiguousarray(w13[c * E_LOC : (c + 1) * E_LOC]),
                "w2Q": w2r,
                "wsgu": wsgu,
                "wsd": wsd,
            }
        )
    return in_maps


last_result = None


def kernel(**inputs):
    global _cached, last_result
    trace = bool(inputs.pop("_trace", False))
    if _cached is None:
        _cached = _build()
    nc = _cached
    in_maps = _prep_inputs(inputs)
    res = run_bass_kernel_spmd(nc, in_maps, core_ids=list(range(N_CORES)), trace=trace)
    last_result = res
    out = np.concatenate([res.results[c]["out"] for c in range(N_CORES)], axis=0).astype(np.float32)
    return np.ascontiguousarray(out)



# revision 20
# speedup vs baseline: 1.5952x; 1.5952x over previous
"""DeepSeek-V3 MoE layer on 8 Trainium2 NeuronCores.

Strategy (expert-parallel + shared-expert tensor-parallel):
  - 64 routed experts sharded 8-per-core; every core computes the dense
    per-expert gated MLP for all 256 tokens of its 8 experts in bf16 and
    accumulates  sum_e cw[t,e] * expert_e(x)[t]  into PSUM.
  - the shared GatedMLP is tensor-parallel on the intermediate dim
    (2048/8 = 256 rows per core) and accumulates into the same PSUM.
  - the noaux-tc sigmoid routing is replicated on every core; the gate
    matmul runs as three bf16 split-precision products (hi*hi + hi*lo +
    lo*hi reproduces fp32 logits to ~1e-5) and is emitted FIRST so the
    DVE routing chain overlaps the shared/expert GEMMs.
  - a ReduceScatter over the [256, 1024] partial output sums the 8
    cores; each core returns its 32-row shard.

Schedule notes (what made this fast):
  - One DMA stream on the sync ring in exact consumption order:
    x/gate/shared-gate weights, then w13 for experts 0..7, then wsd,
    then the four w2 quarters.  Total weight traffic (~25MB/core) is
    near co-critical with PE, so stream order is what keeps PE fed.
  - Routing combine-weights are NOT multiplied into the GEMM1 act
    (which would stall PE behind the DVE routing chain); acts are
    written as silu(h1)*h3 and rescaled in-place per expert once the
    cb broadcast is ready.
  - The down-projection accumulation is region-major: all 34 matmuls
    that accumulate one [128, 512] output region are one contiguous
    start..stop group (interleaving open groups on a shared PSUM bank
    corrupts early contributions on hardware).
"""

import sys

sys.path.insert(0, "/opt/trn_rl_repo")

import numpy as np
import ml_dtypes

import concourse.bacc as bacc
import concourse.mybir as mybir
import concourse.tile as tile
from concourse.bass_utils import run_bass_kernel_spmd

T = 256
H = 1024
E = 64
I = 512
SI = 2048
TOP_K = 6
N_GROUP = 8
TOPK_GROUP = 4
ROUTED_SCALE = 2.5
N_CORES = 8
E_LOC = E // N_CORES          # 8 experts per core
SI_LOC = SI // N_CORES        # 256 shared-intermediate rows per core
KH = H // 128                 # 8 k-tiles over hidden
KI = I // 128                 # 4 k-tiles over routed intermediate
KS = SI_LOC // 128            # 2 k-tiles over local shared intermediate
G2 = 2 * N_GROUP              # 16 groups across both token tiles
J = E // N_GROUP              # 8 experts per group

F32 = mybir.dt.float32
BF16 = mybir.dt.bfloat16
NEG = -1.0e9

_cached = None


def _build():
    nc = bacc.Bacc("TRN2", target_bir_lowering=False, debug=False, num_devices=N_CORES)

    xhi_in = nc.declare_dram_parameter("xhi", [128, KH * T], BF16, isOutput=False)
    xlo_in = nc.declare_dram_parameter("xlo", [128, KH * T], BF16, isOutput=False)
    ghi_in = nc.declare_dram_parameter("ghi", [128, KH * E], BF16, isOutput=False)
    glo_in = nc.declare_dram_parameter("glo", [128, KH * E], BF16, isOutput=False)
    eb_in = nc.declare_dram_parameter("ebias2", [128, 2 * E], F32, isOutput=False)
    sel_in = nc.declare_dram_parameter("sel", [E, E_LOC], BF16, isOutput=False)
    id_in = nc.declare_dram_parameter("identb", [128, 128], BF16, isOutput=False)
    oneh_in = nc.declare_dram_parameter("oneh", [E_LOC, E_LOC * 128], BF16, isOutput=False)
    w13_in = nc.declare_dram_parameter("w13T", [E_LOC, 128, KH * 2 * I], BF16, isOutput=False)
    # per-output-h-tile slabs: [ht, p(i-in-ki), e*KI*128 + ki*128 + hh]
    w2_in = nc.declare_dram_parameter("w2Q", [4, 128, 4 * KI * 512], BF16, isOutput=False)
    wsgu_in = nc.declare_dram_parameter("wsgu", [128, KH * 2 * SI_LOC], BF16, isOutput=False)
    wsd_in = nc.declare_dram_parameter("wsd", [128, KS * H], BF16, isOutput=False)
    out_p = nc.declare_dram_parameter("out", [T // N_CORES, H], BF16, isOutput=True)

    with tile.TileContext(nc) as tc:
        with (
            tc.tile_pool(name="sbuf", bufs=1) as sbuf,
            tc.tile_pool(name="wpool", bufs=4) as wpool,
            tc.tile_pool(name="w2pool", bufs=4) as w2pool,
            tc.tile_pool(name="spsum", bufs=1, space="PSUM") as spsum,
            tc.tile_pool(name="hpsum", bufs=3, space="PSUM") as hpsum,
            tc.tile_pool(name="opsum", bufs=1, space="PSUM") as opsum,
            tc.tile_pool(name="dram", bufs=1, space="DRAM") as dram,
        ):
            # ---- collective warm-up: tiny RS with no compute deps; pays the
            # per-execution collective bring-up + absorbs cross-core launch
            # skew while the real work happens.  Its result is written into a
            # corner of the output (before the real output DMA) so it is
            # never dead code.
            pre_sb = sbuf.tile([16, 16], BF16)
            nc.gpsimd.memset(pre_sb[:], 0.0)
            pre_in = dram.tile([16, 16], BF16)
            pre_out = dram.tile([2, 16], BF16)
            nc.gpsimd.dma_start(pre_in[:], pre_sb[:])
            nc.gpsimd.collective_compute(
                "ReduceScatter",
                mybir.AluOpType.add,
                replica_groups=[list(range(N_CORES))],
                ins=[pre_in.opt()],
                outs=[pre_out.opt()],
            )
            nc.gpsimd.dma_start(out_p[0:2, 0:16], pre_out[:])

            # ---- front loads, sync ring, in consumption order; expert 0's
            # first i-quarter of w13 is pulled forward so GEMM1 can start
            # before the shared-expert weights finish streaming
            xhi = sbuf.tile([128, KH * T], BF16)
            xlo = sbuf.tile([128, KH * T], BF16)
            ghi = sbuf.tile([128, KH * E], BF16)
            glo = sbuf.tile([128, KH * E], BF16)
            wsgu_sb = sbuf.tile([128, KH * 2 * SI_LOC], BF16)
            w13_0 = wpool.tile([128, KH * 2 * I], BF16, tag="w13", name="w13_0")
            QW = KH * 2 * 128          # columns per i-quarter in i-major layout
            SW = KH * 2 * 128          # columns per si-half in si-major wsgu
            nc.sync.dma_start(xhi[:, 0 : 4 * T], xhi_in[:, 0 : 4 * T])
            nc.sync.dma_start(ghi[:], ghi_in[:])
            nc.sync.dma_start(xhi[:, 4 * T :], xhi_in[:, 4 * T :])
            nc.sync.dma_start(xlo[:], xlo_in[:])
            nc.sync.dma_start(glo[:], glo_in[:])
            nc.sync.dma_start(w13_0[:, 0:QW], w13_in[0, :, 0:QW])
            nc.sync.dma_start(wsgu_sb[:, 0:SW], wsgu_in[:, 0:SW])
            nc.sync.dma_start(wsgu_sb[:, SW:], wsgu_in[:, SW:])
            # small tensors on the scalar ring
            eb_sb = sbuf.tile([128, 2 * E], F32)
            nc.scalar.dma_start(eb_sb[:], eb_in[:])
            sel_sb = sbuf.tile([E, E_LOC], BF16)
            nc.scalar.dma_start(sel_sb[:], sel_in[:])
            id_sb = sbuf.tile([128, 128], BF16)
            nc.scalar.dma_start(id_sb[:], id_in[:])
            oneh_sb = sbuf.tile([E_LOC, E_LOC * 128], BF16)
            nc.scalar.dma_start(oneh_sb[:], oneh_in[:])

            # ---- PE p-state warmup: dummy matmuls on a zeroed tile while the
            # first operands stream in, so the real matmuls start at full clock
            warm = sbuf.tile([128, 128], BF16)
            nc.vector.memset(warm[:], 0.0)
            wps = hpsum.tile([128, 2 * T], F32, tag="h13", name="warm")
            for w in range(12):
                nc.tensor.matmul(
                    wps[:, 0:128],
                    warm[:],
                    warm[:],
                    start=(w == 0),
                    stop=(w == 11),
                )
            junk = sbuf.tile([128, 128], BF16)
            nc.vector.tensor_copy(junk[:], wps[:, 0:128])

            # ---- gate logits, split-precision bf16 (fp32-accurate), PE-first
            lp = spsum.tile([128, 2 * E], F32, tag="small", name="logits")
            for tt in range(2):
                reg = lp[:, tt * E : (tt + 1) * E]
                n_mm = 3 * KH
                m = 0
                for xs, gs in ((xhi, ghi), (xlo, ghi), (xhi, glo)):
                    for k in range(KH):
                        nc.tensor.matmul(
                            reg,
                            xs[:, k * T + tt * 128 : k * T + tt * 128 + 128],
                            gs[:, k * E : (k + 1) * E],
                            start=(m == 0),
                            stop=(m == n_mm - 1),
                        )
                        m += 1

            # ---- routing (ACT sigmoid + DVE chain), both token tiles in one
            # [128, 2E] pass; fp32 throughout so the top-k matches reference
            scores = sbuf.tile([128, 2 * E], F32, tag="scores")
            nc.scalar.activation(scores[:], lp[:], mybir.ActivationFunctionType.Sigmoid)
            swb = sbuf.tile([128, 2 * E], F32, tag="swb")
            nc.vector.tensor_add(swb[:], scores[:], eb_sb[:])
            swb3 = swb[:].rearrange("p (G j) -> p G j", G=G2)
            m1 = sbuf.tile([128, G2], F32, tag="m1")
            nc.vector.reduce_max(m1[:], swb3, axis=mybir.AxisListType.X)
            eqt = sbuf.tile([128, 2 * E], F32, tag="eqt")
            nc.vector.tensor_tensor(
                eqt[:].rearrange("p (G j) -> p G j", G=G2),
                swb3,
                m1[:].to_broadcast((128, G2, J)),
                op=mybir.AluOpType.is_equal,
            )
            swb2 = sbuf.tile([128, 2 * E], F32, tag="swb2")
            nc.vector.scalar_tensor_tensor(
                swb2[:], eqt[:], NEG, swb[:],
                op0=mybir.AluOpType.mult, op1=mybir.AluOpType.add,
            )
            m2 = sbuf.tile([128, G2], F32, tag="m2")
            nc.vector.reduce_max(
                m2[:], swb2[:].rearrange("p (G j) -> p G j", G=G2),
                axis=mybir.AxisListType.X,
            )
            gsum = sbuf.tile([128, G2], F32, tag="gsum")
            nc.vector.tensor_add(gsum[:], m1[:], m2[:])
            gsum3 = gsum[:].rearrange("p (t g) -> p t g", t=2)
            gmask = sbuf.tile([128, G2], F32, tag="gmask")
            nc.vector.memset(gmask[:], 0.0)
            for _ in range(TOPK_GROUP):
                gm = sbuf.tile([128, 2], F32, tag="gm")
                nc.vector.reduce_max(gm[:], gsum3, axis=mybir.AxisListType.X)
                geq = sbuf.tile([128, G2], F32, tag="geq")
                nc.vector.tensor_tensor(
                    geq[:].rearrange("p (t g) -> p t g", t=2),
                    gsum3,
                    gm[:].to_broadcast((128, 2, N_GROUP)),
                    op=mybir.AluOpType.is_equal,
                )
                nc.vector.tensor_add(gmask[:], gmask[:], geq[:])
                nc.vector.scalar_tensor_tensor(
                    gsum[:], geq[:], NEG, gsum[:],
                    op0=mybir.AluOpType.mult, op1=mybir.AluOpType.add,
                )
            swbm = sbuf.tile([128, 2 * E], F32, tag="swbm")
            nc.vector.tensor_tensor(
                swbm[:].rearrange("p (G j) -> p G j", G=G2),
                swb3,
                gmask[:].to_broadcast((128, G2, J)),
                op=mybir.AluOpType.mult,
            )
            swbm3 = swbm[:].rearrange("p (t e) -> p t e", t=2)
            nmask = sbuf.tile([128, 2 * E], F32, tag="nmask")
            nc.vector.memset(nmask[:], 0.0)
            for _ in range(TOP_K):
                em = sbuf.tile([128, 2], F32, tag="em")
                nc.vector.reduce_max(em[:], swbm3, axis=mybir.AxisListType.X)
                eeq = sbuf.tile([128, 2 * E], F32, tag="eeq")
                nc.vector.tensor_tensor(
                    eeq[:].rearrange("p (t e) -> p t e", t=2),
                    swbm3,
                    em[:].to_broadcast((128, 2, E)),
                    op=mybir.AluOpType.is_equal,
                )
                nc.vector.tensor_add(nmask[:], nmask[:], eeq[:])
                nc.vector.scalar_tensor_tensor(
                    swbm[:], eeq[:], NEG, swbm[:],
                    op0=mybir.AluOpType.mult, op1=mybir.AluOpType.add,
                )
            s_sb = sbuf.tile([128, 2 * E], F32, tag="s_sb")
            nc.vector.tensor_mul(s_sb[:], scores[:], nmask[:])
            denom = sbuf.tile([128, 2], F32, tag="denom")
            nc.vector.reduce_sum(
                denom[:], s_sb[:].rearrange("p (t e) -> p t e", t=2),
                axis=mybir.AxisListType.X,
            )
            dr = sbuf.tile([128, 2], F32, tag="dr")
            nc.vector.reciprocal(dr[:], denom[:])
            cw_sb = sbuf.tile([128, 2 * E], BF16)
            nc.vector.scalar_tensor_tensor(
                cw_sb[:].rearrange("p (t e) -> p t e", t=2),
                s_sb[:].rearrange("p (t e) -> p t e", t=2),
                ROUTED_SCALE,
                dr[:].to_broadcast((128, 2, E)),
                op0=mybir.AluOpType.mult, op1=mybir.AluOpType.mult,
            )

            # ---- routed experts GEMM1 + act (acts WITHOUT combine weights)
            act_sbs = []

            def gemm1_dma(e):
                if e == 0:
                    w13_sb = w13_0
                    q0 = 1
                else:
                    w13_sb = wpool.tile([128, KH * 2 * I], BF16, tag="w13", name=f"w13_{e}")
                    q0 = 0
                for q in range(q0, 4):
                    nc.sync.dma_start(
                        w13_sb[:, q * QW : (q + 1) * QW],
                        w13_in[e, :, q * QW : (q + 1) * QW],
                    )
                act_sb = sbuf.tile([128, KI * T], BF16, tag=f"act{e}", name=f"act{e}")
                act_sbs.append(act_sb)
                return w13_sb

            def gemm1_tiles(e, w13_sb, i_lo, i_hi):
                # w13 is i-major: [i][k][gate 128 | up 128]; each i-quarter is
                # one DMA so i-tile i's matmuls start as soon as it lands
                act_sb = act_sbs[e]
                for i in range(i_lo, i_hi):
                    hp = hpsum.tile([128, 2 * T], F32, tag="h13", name=f"h13_{e}_{i}")
                    for k in range(KH):
                        nc.tensor.matmul(
                            hp[:, 0:T],
                            w13_sb[:, i * QW + k * 256 : i * QW + k * 256 + 128],
                            xhi[:, k * T : (k + 1) * T],
                            start=(k == 0),
                            stop=(k == KH - 1),
                        )
                    for k in range(KH):
                        nc.tensor.matmul(
                            hp[:, T : 2 * T],
                            w13_sb[:, i * QW + k * 256 + 128 : i * QW + k * 256 + 256],
                            xhi[:, k * T : (k + 1) * T],
                            start=(k == 0),
                            stop=(k == KH - 1),
                        )
                    sl = sbuf.tile([128, T], BF16, tag="sl")
                    nc.scalar.activation(sl[:], hp[:, 0:T], mybir.ActivationFunctionType.Silu)
                    nc.vector.tensor_mul(act_sb[:, i * T : (i + 1) * T], sl[:], hp[:, T : 2 * T])

            def rescale_expert(e):
                # multiply the per-token combine weight for this expert into
                # its act tile (in place), once cb is ready
                for i in range(KI):
                    nc.vector.tensor_mul(
                        act_sbs[e][:, i * T : (i + 1) * T],
                        act_sbs[e][:, i * T : (i + 1) * T],
                        cb_sb[:, e * T : (e + 1) * T],
                    )

            # expert 0, first i-tile: runs while the shared weights stream
            w13_e0 = gemm1_dma(0)
            gemm1_tiles(0, w13_e0, 0, 1)

            # ---- shared expert gate/up GEMMs (wsgu is si-major like w13)
            su = []
            for si in range(KS):
                sp = hpsum.tile([128, 2 * T], F32, tag="h13", name=f"su{si}")
                for k in range(KH):
                    nc.tensor.matmul(
                        sp[:, 0:T],
                        wsgu_sb[:, si * SW + k * 256 : si * SW + k * 256 + 128],
                        xhi[:, k * T : (k + 1) * T],
                        start=(k == 0),
                        stop=(k == KH - 1),
                    )
                for k in range(KH):
                    nc.tensor.matmul(
                        sp[:, T : 2 * T],
                        wsgu_sb[:, si * SW + k * 256 + 128 : si * SW + k * 256 + 256],
                        xhi[:, k * T : (k + 1) * T],
                        start=(k == 0),
                        stop=(k == KH - 1),
                    )
                su.append(sp)
            acts_sh = sbuf.tile([128, KS * T], BF16)
            for si in range(KS):
                ssl = sbuf.tile([128, T], BF16, tag="ssl")
                nc.scalar.activation(ssl[:], su[si][:, 0:T], mybir.ActivationFunctionType.Silu)
                nc.vector.tensor_mul(acts_sh[:, si * T : (si + 1) * T], ssl[:], su[si][:, T : 2 * T])

            # expert 0, remaining i-tiles
            gemm1_tiles(0, w13_e0, 1, KI)

            # ---- per-expert combine weights: cb[j] = broadcast of
            # cw[:, core*8+j] across all 128 partitions (bf16 throughout)
            cwT_sb = sbuf.tile([E, T], BF16)
            for tt in range(2):
                ctp = spsum.tile([E, 128], BF16, tag="small", name=f"ctp{tt}")
                nc.tensor.transpose(ctp[:], cw_sb[:, tt * E : (tt + 1) * E], id_sb[:])
                nc.vector.tensor_copy(cwT_sb[:, tt * 128 : (tt + 1) * 128], ctp[:])
            cwl_ps = spsum.tile([E_LOC, T], F32, tag="small")
            nc.tensor.matmul(cwl_ps[:], sel_sb[:], cwT_sb[:], start=True, stop=True)
            cwl_sb = sbuf.tile([E_LOC, T], BF16)
            nc.vector.tensor_copy(cwl_sb[:], cwl_ps[:])
            cb_sb = sbuf.tile([128, E_LOC * T], BF16)
            for j in range(E_LOC):
                cbp = spsum.tile([128, T], F32, tag="small", name=f"cbp{j}")
                nc.tensor.matmul(
                    cbp[:], oneh_sb[:, j * 128 : (j + 1) * 128], cwl_sb[:],
                    start=True, stop=True,
                )
                nc.vector.tensor_copy(cb_sb[:, j * T : (j + 1) * T], cbp[:])

            rescale_expert(0)
            for e in range(1, E_LOC):
                w13_sb = gemm1_dma(e)
                gemm1_tiles(e, w13_sb, 0, KI)
                rescale_expert(e)

            # ---- down-projections: act tiles stationary, w2 streams as the
            # wide (N=512) moving operand; 4 accumulation regions (tt, hh),
            # each one closed start..stop group in its own PSUM bank.
            wsd_sb = sbuf.tile([128, KS * H], BF16)
            nc.sync.dma_start(wsd_sb[:], wsd_in[:])
            out_ps = [opsum.tile([128, H], F32, tag=f"out{tt}", name=f"out{tt}") for tt in range(2)]
            outf = sbuf.tile([128, 2 * H], BF16)
            rs_in = dram.tile([T, H], BF16)
            rs_out = dram.tile([T // N_CORES, H], BF16)

            w2q = {}
            for q in range(4):
                w2q[q] = w2pool.tile([128, 4 * KI * 512], BF16, tag="w2q", name=f"w2q{q}")
                nc.sync.dma_start(w2q[q][:], w2_in[q, :, :])

            # each region's accumulation group is split at the w2-quarter
            # boundary and the halves interleaved across the two token-tile
            # PSUM banks, so the last-arriving quarter gates the least work.
            # groups on DIFFERENT banks may interleave; each region's own
            # start..stop sequence stays in order.
            def gemm2_half(hh, tt, eh):
                reg = out_ps[tt][:, hh * 512 : (hh + 1) * 512]
                if eh == 0:
                    for ks in range(KS):
                        nc.tensor.matmul(
                            reg,
                            acts_sh[:, ks * T + tt * 128 : ks * T + tt * 128 + 128],
                            wsd_sb[:, ks * H + hh * 512 : ks * H + (hh + 1) * 512],
                            start=(ks == 0),
                            stop=False,
                        )
                qt = w2q[hh * 2 + eh]
                for er in range(4):
                    e = eh * 4 + er
                    for ki in range(KI):
                        nc.tensor.matmul(
                            reg,
                            act_sbs[e][:, ki * T + tt * 128 : ki * T + tt * 128 + 128],
                            qt[:, (er * KI + ki) * 512 : (er * KI + ki) * 512 + 512],
                            start=False,
                            stop=(e == E_LOC - 1 and ki == KI - 1),
                        )

            for hh in range(2):
                for eh in range(2):
                    for tt in range(2):
                        gemm2_half(hh, tt, eh)
                    if hh == 1 and eh == 1:
                        for tt in range(2):
                            nc.vector.tensor_copy(outf[:, tt * H : (tt + 1) * H], out_ps[tt][:])
                            nc.sync.dma_start(rs_in[tt * 128 : (tt + 1) * 128, :], outf[:, tt * H : (tt + 1) * H])

            # ---- ReduceScatter over cores: each core gets 32 tokens x H
            nc.gpsimd.collective_compute(
                "ReduceScatter",
                mybir.AluOpType.add,
                replica_groups=[list(range(N_CORES))],
                ins=[rs_in.opt()],
                outs=[rs_out.opt()],
            )
            nc.sync.dma_start(out_p[:], rs_out[:])

    nc.finalize()
    return nc


def _prep_inputs(inputs):
    bf = ml_dtypes.bfloat16
    x = np.asarray(inputs["hidden_states"], np.float32)
    gate_w = np.asarray(inputs["gate_w"], np.float32)
    e_bias = np.asarray(inputs["e_bias"], np.float32)
    w1 = np.asarray(inputs["w1"], np.float32)
    w3 = np.asarray(inputs["w3"], np.float32)
    w2 = np.asarray(inputs["w2"], np.float32)
    ws_gate = np.asarray(inputs["ws_gate"], np.float32)
    ws_up = np.asarray(inputs["ws_up"], np.float32)
    ws_down = np.asarray(inputs["ws_down"], np.float32)

    xT = np.ascontiguousarray(x.T.reshape(KH, 128, T).transpose(1, 0, 2).reshape(128, KH * T))
    xhi = xT.astype(bf)
    xlo = (xT - xhi.astype(np.float32)).astype(bf)
    gT = np.ascontiguousarray(gate_w.T.reshape(KH, 128, E).transpose(1, 0, 2).reshape(128, KH * E))
    ghi = gT.astype(bf)
    glo = (gT - ghi.astype(np.float32)).astype(bf)
    ebb2 = np.broadcast_to(np.tile(e_bias, 2)[None, :], (128, 2 * E)).copy()
    identb = np.eye(128, dtype=np.float32).astype(bf)
    oneh = np.zeros((E_LOC, E_LOC * 128), np.float32)
    for j in range(E_LOC):
        oneh[j, j * 128 : (j + 1) * 128] = 1.0
    oneh = oneh.astype(bf)

    # routed up/gate weights, i-major: w13[e, p, i*KH*256 + k*256 + (g|u)*128 + ii]
    w1t = w1.transpose(0, 2, 1).reshape(E, KH, 128, KI, 128)   # [e, k, p, i, ii]
    w3t = w3.transpose(0, 2, 1).reshape(E, KH, 128, KI, 128)
    w13 = np.stack([w1t, w3t], axis=4)                         # [e, k, p, i, gu, ii]
    w13 = w13.transpose(0, 2, 3, 1, 4, 5).reshape(E, 128, KI * KH * 2 * 128).astype(bf)
    # routed down weights as rhs quarters:
    # w2Q[c][hh*2+eh, p, ((er*KI)+ki)*512 + hc] = w2[8c+4*eh+er][hh*512+hc, ki*128+p]
    w2t = w2.transpose(0, 2, 1).reshape(E, KI, 128, 2, 512)   # [e, ki, p, hh, hc]
    w2t = w2t.transpose(0, 3, 2, 1, 4)                        # [e, hh, p, ki, hc]

    in_maps = []
    for c in range(N_CORES):
        sel = np.zeros((E, E_LOC), np.float32)
        for j in range(E_LOC):
            sel[c * E_LOC + j, j] = 1.0
        # si-major: wsgu[p, si*KH*256 + k*256 + (g|u)*128 + ss]
        wsg = ws_gate[c * SI_LOC : (c + 1) * SI_LOC, :].T.reshape(KH, 128, KS, 128)
        wsu = ws_up[c * SI_LOC : (c + 1) * SI_LOC, :].T.reshape(KH, 128, KS, 128)
        wsgu = np.stack([wsg, wsu], axis=3)                   # [k, p, si, gu, ss]
        wsgu = wsgu.transpose(1, 2, 0, 3, 4).reshape(128, KS * KH * 2 * 128).astype(bf)
        wsd = ws_down[:, c * SI_LOC : (c + 1) * SI_LOC].T.reshape(KS, 128, H)
        wsd = wsd.transpose(1, 0, 2).reshape(128, KS * H).astype(bf)
        wc = w2t[c * E_LOC : (c + 1) * E_LOC]                 # [8, hh, p, ki, hc]
        wc = wc.reshape(2, 4, 2, 128, KI, 512)                # [eh, er, hh, p, ki, hc]
        wc = wc.transpose(2, 0, 3, 1, 4, 5)                   # [hh, eh, p, er, ki, hc]
        w2r = np.ascontiguousarray(wc.reshape(4, 128, 4 * KI * 512)).astype(bf)
        in_maps.append(
            {
                "xhi": xhi,
                "xlo": xlo,
                "ghi": ghi,
                "glo": glo,
                "ebias2": ebb2,
                "sel": sel.astype(bf),
                "identb": identb,
                "oneh": oneh,
                "w13T": np.ascontiguousarray(w13[c * E_LOC : (c + 1) * E_LOC]),
                "w2Q": w2r,
                "wsgu": wsgu,
                "wsd": wsd,
            }
        )
    return in_maps


last_result = None


def kernel(**inputs):
    global _cached, last_result
    trace = bool(inputs.pop("_trace", False))
    if _cached is None:
        _cached = _build()
    nc = _cached
    in_maps = _prep_inputs(inputs)
    res = run_bass_kernel_spmd(nc, in_maps, core_ids=list(range(N_CORES)), trace=trace)
    last_result = res
    out = np.concatenate([res.results[c]["out"] for c in range(N_CORES)], axis=0).astype(np.float32)
    return np.ascontiguousarray(out)
